# revision 1
# baseline (speedup 1.0000x reference)
"""LoRA linear on 8 Trainium2 NeuronCores.

out = x @ (W + A @ B)^T + bias
  x: [4, 4096, 4096] f32, W: [4096, 4096], bias: [4096], A: [4096, 16], B: [16, 4096]

Strategy: 2D shard (2 row-shards x 4 col-shards), fp32r matmuls, fp32 PSUM.
  - Host: Weff = W + A@B (0.1% of FLOPs), pre-transpose + pre-block x and Weff
    so the contraction dim lands on SBUF partitions with no on-chip
    transposes. W is rounded to FP32R (fp32 with 11-bit mantissa): the PE
    runs fp32r at 1 cycle/row with the self-loading weight path (bf16 pays a
    non-overlapped LdWeights bubble per matmul; fp32r does not).
  - x streams from HBM as bf16 (half the bytes) and is cast on-chip by the
    vector engine into an f32 tile, bitcast to fp32r for the matmul. The
    cast is fully hidden under the matmul chains.
  - Core c=(mi,nj): out[mi*8192:(mi+1)*8192, nj*1024:(nj+1)*1024] =
    x_rows(mi) @ WeffT_cols(nj) + bias(nj).  W shard (16.8 MB fp32r) is
    SBUF-resident; x row-shard streams in m-blocks; 32 k-tile matmuls
    accumulate in PSUM; bias add fused into the PSUM->SBUF evacuation.
  - vs. the column-parallel baseline this cuts per-core input DMA from
    ~276 MB (x fully replicated fp32) to ~84 MB and total HBM traffic from
    ~2.5 GB to ~0.9 GB, so the kernel stays PE-bound even when the shared
    HBM/DMA path is contended.
"""
import numpy as np
import ml_dtypes

import concourse.bacc as bacc
import concourse.mybir as mybir
import concourse.tile as tile
from concourse.bass_utils import run_bass_kernel_spmd

BATCH, SEQ, D = 4, 4096, 4096
M = BATCH * SEQ          # 16384 rows
K = D                    # contraction
N_CORES = 8
MSH, NSH = 2, 4          # core grid: 2 row-shards x 4 col-shards
MC = M // MSH            # 8192 rows per core
NC = D // NSH            # 1024 output cols per core
KT = K // 128            # 32 k-tiles
NB = 512                 # psum tile width (n-chunk)
NCH = NC // NB           # 2 n-chunks
MB = 128                 # m-block rows per x stream tile
XBUFS = 2                # x f32 tile double-buffering depth
SBUFS = 2                # x bf16 staging buffers

_f32 = mybir.dt.float32
_f32r = mybir.dt.float32r
_bf16 = mybir.dt.bfloat16
_bf16np = ml_dtypes.bfloat16

_COMPILED = None


def _build(repeat=1):
    """repeat>1 wraps the compute in a For_i loop that redundantly recomputes
    the same output -- used only for marginal-cost HW timing (the axon
    dispatch floor is ~80ms, far above the ~1ms kernel)."""
    import contextlib
    nc = bacc.Bacc("TRN2", target_bir_lowering=False, debug=False,
                   num_devices=N_CORES)
    # x pre-blocked on host: [mb, kt*128+p, j] contiguous per (mb, kt) tile
    xT = nc.dram_tensor("xT", [MC // MB, KT * 128, MB], _bf16,
                        kind="ExternalInput").ap()
    # W pre-blocked on host: [(nch*KT+kt)*128+p, j]
    wT = nc.dram_tensor("wT", [NCH * KT * 128, NB], _f32r,
                        kind="ExternalInput").ap()
    bias = nc.dram_tensor("bias", [128, NC], _f32, kind="ExternalInput").ap()
    out = nc.dram_tensor("out", [MC, NC], _f32, kind="ExternalOutput").ap()

    with tile.TileContext(nc) as tc:
        with tc.tile_pool(name="w", bufs=1) as wp, \
             tc.tile_pool(name="xs", bufs=SBUFS) as sxp, \
             tc.tile_pool(name="xb", bufs=XBUFS) as xp, \
             tc.tile_pool(name="ob", bufs=3) as op_, \
             tc.tile_pool(name="ps", bufs=4, space="PSUM") as pp:
            # W shard resident, one tile per n-chunk so the first m-block's
            # matmuls only wait on nch0's 8.4 MB, not the full 16.8 MB load
            w_sb = []
            for nch in range(NCH):
                t = wp.tile([128, KT * NB], _f32r, tag=f"w{nch}")
                nc.sync.dma_start(
                    out=t[:].rearrange("p (t j) -> p t j", j=NB),
                    in_=wT[nch * KT * 128:(nch + 1) * KT * 128, :]
                        .rearrange("(t p) j -> p t j", p=128))
                w_sb.append(t)
            b_sb = wp.tile([128, NC], _f32, tag="bias")
            nc.sync.dma_start(out=b_sb[:], in_=bias)

            loop_cm = (tc.For_i(0, repeat, 1) if repeat > 1
                       else contextlib.nullcontext())
            with loop_cm:
                _emit_body(nc, tc, sxp, xp, op_, pp, xT, out, w_sb, b_sb)

    nc.compile()
    return nc


def _emit_body(nc, tc, sxp, xp, op_, pp, xT, out, w_sb, b_sb):
    for mb in range(MC // MB):
        xs = sxp.tile([128, KT * MB], _bf16, tag="xs")
        # one DMA per block (host pre-blocked x): src [p, kt, j]
        nc.sync.dma_start(
            out=xs[:].rearrange("p (kt j) -> p kt j", j=MB),
            in_=xT[mb].rearrange("(kt p) j -> p kt j", p=128))
        # upcast bf16 -> fp32r on gpsimd (exact; hidden under PE, and off the
        # DVE queue so it doesn't serialize behind the bias adds)
        xt = xp.tile([128, KT * MB], _f32r, tag="x")
        nc.gpsimd.tensor_copy(xt[:], xs[:])
        o_sb = op_.tile([128, NC], _f32, tag="o")
        for nch in range(NCH):
            ps = pp.tile([128, NB], _f32, tag="acc")
            for kt in range(KT):
                nc.tensor.matmul(
                    ps[:],
                    xt[:, kt * MB:kt * MB + 128],
                    w_sb[nch][:, kt * NB:(kt + 1) * NB],
                    start=(kt == 0), stop=(kt == KT - 1))
            nc.vector.tensor_add(o_sb[:, nch * NB:(nch + 1) * NB], ps[:],
                                 b_sb[:, nch * NB:(nch + 1) * NB])
        row = mb * MB
        nc.sync.dma_start(out=out[row:row + 128, :], in_=o_sb[:])


def _compiled():
    global _COMPILED
    if _COMPILED is None:
        _COMPILED = _build()
    return _COMPILED


def _round_fp32r_inplace(a):
    """Round fp32 array to FP32R (round-to-nearest-even to 11 mantissa bits,
    low 12 bits zeroed). Safe for finite data."""
    u = a.view(np.uint32)
    lsb = (u >> 12) & np.uint32(1)
    u += np.uint32(0x7FF)
    u += lsb
    u &= np.uint32(0xFFFFF000)
    return a


def _prep_in_maps(x, W, bias, A, B):
    x = np.asarray(x, dtype=np.float32).reshape(M, K)
    W = np.asarray(W, dtype=np.float32)
    bias = np.asarray(bias, dtype=np.float32)
    A = np.asarray(A, dtype=np.float32)
    B = np.asarray(B, dtype=np.float32)

    weff_t = (W + A @ B).T.copy()            # [K, D] k-major
    _round_fp32r_inplace(weff_t)
    # per col-shard: block to [(nch kt p), j]
    w_blocks = []
    for nj in range(NSH):
        wsl = np.ascontiguousarray(
            weff_t[:, nj * NC:(nj + 1) * NC]
            .reshape(KT, 128, NCH, NB).transpose(2, 0, 1, 3)
        ).reshape(NCH * KT * 128, NB)
        w_blocks.append(wsl)
    # per row-shard: k-major blocked [mb, kt*128+p, j], bf16
    x_blocks = []
    for mi in range(MSH):
        xs = np.ascontiguousarray(
            x[mi * MC:(mi + 1) * MC, :].T).astype(_bf16np)    # [K, MC]
        xs = np.ascontiguousarray(
            xs.reshape(KT, 128, MC // MB, MB).transpose(2, 0, 1, 3)
        ).reshape(MC // MB, KT * 128, MB)
        x_blocks.append(xs)
    b_rows = [np.tile(bias[nj * NC:(nj + 1) * NC], (128, 1))
              for nj in range(NSH)]

    in_maps = []
    for c in range(N_CORES):
        mi, nj = divmod(c, NSH)
        in_maps.append({
            "xT": x_blocks[mi],
            "wT": w_blocks[nj],
            "bias": b_rows[nj],
        })
    return in_maps


def kernel(x, W, bias, A, B):
    nc = _compiled()
    in_maps = _prep_in_maps(x, W, bias, A, B)
    res = run_bass_kernel_spmd(nc, in_maps, core_ids=list(range(N_CORES)),
                               trace=False)
    out = np.empty((M, D), dtype=np.float32)
    for c in range(N_CORES):
        mi, nj = divmod(c, NSH)
        out[mi * MC:(mi + 1) * MC, nj * NC:(nj + 1) * NC] = \
            res.results[c]["out"]
    return out.reshape(BATCH, SEQ, D)



# revision 2
# speedup vs baseline: 1.0802x; 1.0802x over previous
"""LoRA linear on 8 Trainium2 NeuronCores.

out = x @ (W + A @ B)^T + bias
  x: [4, 4096, 4096] f32, W: [4096, 4096], bias: [4096], A: [4096, 16], B: [16, 4096]

Strategy: 4x2 shard (4 row-shards x 2 col-shards), bf16 matmuls with a
partial fp8 DoubleRow fast path, fp32 PSUM.
  - Host: Weff = W + A@B (0.1% of FLOPs), pre-transpose + pre-block x and
    Weff so the contraction dim lands on SBUF partitions with no on-chip
    transposes.  W shard [4096, 2048] is bf16 SBUF-resident (16.8 MB);
    per-core x is 33.5 MB bf16 (each row-shard read by only 2 cores).
  - fp8 fast path: KF=6 of 32 k-tiles run as fp8e4 DoubleRow matmuls
    (256 contraction rows per pass at ~1.8x the per-FLOP rate; measured
    ~14% end-to-end).  fp8 operands are pre-scaled x*2^-4 / W*2^4 on host
    so the product scale is exactly 1 and accumulates into the same PSUM
    group as the bf16 k-tiles with no on-chip rescale.
  - Accuracy budget: full-output max rel err 1.74e-2 (CPU-generated
    inputs) / 1.18e-2 (device-generated inputs) vs the 2e-2 gate; KF=8
    would exceed the gate (2.03e-2), KF=6 is the ceiling.
  - Loop: per 128-row m-block, k-outer / n-inner with 4 concurrently open
    PSUM accumulation groups; stationary x[kt] amortized over the 4
    n-chunk matmuls.  The DoubleRow block runs LAST so its (slow,
    non-FWL) LDWEIGHTS pull ahead under the bf16 matmul streams
    (measured ~2.6% over DR-first).  Bias add fused into PSUM->SBUF
    evacuation on DVE; x streams double-buffered; out DMA'd per m-block.
"""
import numpy as np
import ml_dtypes

import concourse.bacc as bacc
import concourse.mybir as mybir
import concourse.tile as tile
from concourse.bass_utils import run_bass_kernel_spmd

BATCH, SEQ, D = 4, 4096, 4096
M = BATCH * SEQ          # 16384 rows
K = D                    # contraction
N_CORES = 8
MSH, NSH = 4, 2          # core grid: 4 row-shards x 2 col-shards
MC = M // MSH            # 4096 rows per core
NC = D // NSH            # 2048 output cols per core
KT = K // 128            # 32 k-tiles
KF = 6                   # k-tiles computed in fp8 DoubleRow (must be even)
KST8 = KF // 2           # DR supertiles (256 contraction rows each)
KBF = KT - KF            # bf16 k-tiles
NB = 512                 # psum tile width (n-chunk)
NCH = NC // NB           # 4 n-chunks
MB = 128                 # m-block rows per x stream tile
XS_LOG2, WS_LOG2 = -4, 4  # fp8 pre-scales (product scale = 1)

_f32 = mybir.dt.float32
_bf16 = mybir.dt.bfloat16
_f8 = mybir.dt.float8e4
_bf16np = ml_dtypes.bfloat16
_f8np = ml_dtypes.float8_e4m3fn

_COMPILED = None


def _build(repeat=1):
    import contextlib
    nc = bacc.Bacc("TRN2", target_bir_lowering=False, debug=False,
                   num_devices=N_CORES)
    # fp8 x part: per mb, rows (kst two p), cols MB
    xT8 = nc.dram_tensor("xT8", [MC // MB, KST8 * 2 * 128, MB], _f8,
                         kind="ExternalInput").ap()
    # bf16 x part: per mb, rows (kt p) for kt in KF..KT-1
    xT = nc.dram_tensor("xT", [MC // MB, KBF * 128, MB], _bf16,
                        kind="ExternalInput").ap()
    # fp8 W part: [(kst two p), NC]
    wT8 = nc.dram_tensor("wT8", [KST8 * 2 * 128, NC], _f8,
                         kind="ExternalInput").ap()
    # bf16 W part: [(kt p), NC]
    wT = nc.dram_tensor("wT", [KBF * 128, NC], _bf16,
                        kind="ExternalInput").ap()
    bias = nc.dram_tensor("bias", [128, NC], _f32, kind="ExternalInput").ap()
    out = nc.dram_tensor("out", [MC, NC], _f32, kind="ExternalOutput").ap()

    with tile.TileContext(nc) as tc:
        with tc.tile_pool(name="w", bufs=1) as wp, \
             tc.tile_pool(name="xs", bufs=4) as sxp, \
             tc.tile_pool(name="ob", bufs=3) as op_, \
             tc.tile_pool(name="ps", bufs=2, space="PSUM") as pp:
            w8_sb, w_sb = [], []
            for nch in range(NCH):
                t8 = wp.tile([128, KST8 * 2 * NB], _f8, tag=f"w8{nch}",
                             name=f"w8_{nch}")
                nc.sync.dma_start(
                    out=t8[:].rearrange("p (t j) -> p t j", j=NB),
                    in_=wT8[:, nch * NB:(nch + 1) * NB]
                        .rearrange("(t p) j -> p t j", p=128))
                w8_sb.append(t8)
                t = wp.tile([128, KBF * NB], _bf16, tag=f"w{nch}",
                            name=f"w_{nch}")
                nc.sync.dma_start(
                    out=t[:].rearrange("p (t j) -> p t j", j=NB),
                    in_=wT[:, nch * NB:(nch + 1) * NB]
                        .rearrange("(t p) j -> p t j", p=128))
                w_sb.append(t)
            b_sb = wp.tile([128, NC], _f32, tag="bias")
            nc.sync.dma_start(out=b_sb[:], in_=bias)

            loop_cm = (tc.For_i(0, repeat, 1) if repeat > 1
                       else contextlib.nullcontext())
            with loop_cm:
                _emit_body(nc, tc, sxp, op_, pp, xT8, xT, out,
                           w8_sb, w_sb, b_sb)

    nc.compile()
    return nc


def _emit_body(nc, tc, sxp, op_, pp, xT8, xT, out, w8_sb, w_sb, b_sb):
    for mb in range(MC // MB):
        xs8 = sxp.tile([128, KST8 * 2 * MB], _f8, tag="xs8")
        nc.sync.dma_start(
            out=xs8[:].rearrange("p (t j) -> p t j", j=MB),
            in_=xT8[mb].rearrange("(t p) j -> p t j", p=128))
        xs = sxp.tile([128, KBF * MB], _bf16, tag="xs")
        nc.sync.dma_start(
            out=xs[:].rearrange("p (kt j) -> p kt j", j=MB),
            in_=xT[mb].rearrange("(kt p) j -> p kt j", p=128))
        o_sb = op_.tile([128, NC], _f32, tag="o")
        ps = [pp.tile([128, NB], _f32, tag=f"acc{n}", name=f"ps{n}")
              for n in range(NCH)]
        for kt in range(KBF):
            for nch in range(NCH):
                nc.tensor.matmul(
                    ps[nch][:],
                    xs[:, kt * MB:kt * MB + 128],
                    w_sb[nch][:, kt * NB:(kt + 1) * NB],
                    start=(kt == 0), stop=False)
        for kst in range(KST8):
            for nch in range(NCH):
                nc.tensor.matmul(
                    ps[nch][:],
                    xs8[:, kst * 2 * MB:(kst + 1) * 2 * MB]
                        .rearrange("p (two m) -> p two m", two=2),
                    w8_sb[nch][:, kst * 2 * NB:(kst + 1) * 2 * NB]
                        .rearrange("p (two n) -> p two n", two=2),
                    start=False, stop=(kst == KST8 - 1),
                    perf_mode=mybir.MatmulPerfMode.DoubleRow)
        for nch in range(NCH):
            nc.vector.tensor_add(o_sb[:, nch * NB:(nch + 1) * NB],
                                 ps[nch][:],
                                 b_sb[:, nch * NB:(nch + 1) * NB])
        row = mb * MB
        nc.sync.dma_start(out=out[row:row + 128, :], in_=o_sb[:])


def _compiled():
    global _COMPILED
    if _COMPILED is None:
        _COMPILED = _build()
    return _COMPILED


def _prep_in_maps(x, W, bias, A, B):
    x = np.asarray(x, dtype=np.float32).reshape(M, K)
    W = np.asarray(W, dtype=np.float32)
    bias = np.asarray(bias, dtype=np.float32)
    A = np.asarray(A, dtype=np.float32)
    B = np.asarray(B, dtype=np.float32)

    weff_t = (W + A @ B).T                   # [K, D] k-major f32
    kf = KF * 128
    w8_full = (weff_t[:kf] * 2.0 ** WS_LOG2).astype(_f8np)   # [kf, D]
    wbf_full = weff_t[kf:].astype(_bf16np)                   # [K-kf, D]
    w8_blocks, w_blocks = [], []
    for nj in range(NSH):
        w8_blocks.append(np.ascontiguousarray(
            w8_full[:, nj * NC:(nj + 1) * NC]))   # [(kst two p), NC]
        w_blocks.append(np.ascontiguousarray(
            wbf_full[:, nj * NC:(nj + 1) * NC]))  # [(kt p), NC]

    x_blocks8, x_blocks = [], []
    for mi in range(MSH):
        xsh = np.ascontiguousarray(x[mi * MC:(mi + 1) * MC, :].T)  # [K, MC]
        x8 = (xsh[:kf] * 2.0 ** XS_LOG2).astype(_f8np)
        x8 = np.ascontiguousarray(
            x8.reshape(KST8 * 2 * 128, MC // MB, MB).transpose(1, 0, 2))
        x_blocks8.append(x8)                  # [mb, (kst two p), MB]
        xbf = xsh[kf:].astype(_bf16np)
        xbf = np.ascontiguousarray(
            xbf.reshape(KBF * 128, MC // MB, MB).transpose(1, 0, 2))
        x_blocks.append(xbf)                  # [mb, (kt p), MB]
    b_rows = [np.tile(bias[nj * NC:(nj + 1) * NC], (128, 1))
              for nj in range(NSH)]

    in_maps = []
    for c in range(N_CORES):
        mi, nj = divmod(c, NSH)
        in_maps.append({
            "xT8": x_blocks8[mi],
            "xT": x_blocks[mi],
            "wT8": w8_blocks[nj],
            "wT": w_blocks[nj],
            "bias": b_rows[nj],
        })
    return in_maps


def kernel(x, W, bias, A, B):
    nc = _compiled()
    in_maps = _prep_in_maps(x, W, bias, A, B)
    res = run_bass_kernel_spmd(nc, in_maps, core_ids=list(range(N_CORES)),
                               trace=False)
    out = np.empty((M, D), dtype=np.float32)
    for c in range(N_CORES):
        mi, nj = divmod(c, NSH)
        out[mi * MC:(mi + 1) * MC, nj * NC:(nj + 1) * NC] = \
            res.results[c]["out"]
    return out.reshape(BATCH, SEQ, D)
